# revision 21
# baseline (speedup 1.0000x reference)
"""DeepSeekV3 MLA attention kernel for Trainium2 (8 NeuronCores, Bass/Tile).

Sharding: core c -> batch b = c // 4, head-group g = c % 4 (8 of 32 heads).
Each core runs the layer for its batch restricted to its heads and emits a
partial o_proj output [2048, 4096]; the host sums the 4 partials per batch.

The shared a-projections (q_a, kv_a) are *sequence-parallel*: each core
computes and RMS-normalizes the latents for its 512-token slice only, then an
AllGather over the 4-core batch group replicates the full normalized latents
(2.1 MB/rank, bf16) while the per-head k_rope projection keeps the PE busy.
This removes the 4x replicated a-projection compute of the naive layout
(~330 us of PE time per core).

Layouts (feature-major, [128, chunks, tokens]); all matmul operands are bf16
(full PE rate, half the DMA/SBUF of fp32r), PSUM accumulation fp32:
  - x is fed transposed (xT [4096, 2048]); matmuls contract over the
    partition dim with N = 512 token tiles (one PSUM bank).
  - RoPE halves are packed [4*lo(128) | 4*hi(128)] per 4 heads so the rotate
    is partition-aligned full-lane DVE work; a DMA rearrange then stores
    per-head-contiguous [64] blocks that attention contracts over (K=64).
    Head pairs (2j, 2j+1) land in partition halves [0:64], [64:128] of the
    same chunk, so attention row-packs the two K=64 rope matmuls into one
    PE pass via tile_position (concurrent sub-array execution).
  - Softmax skips the max-subtraction (scores are O(5), exp safe in fp32);
    denominators come from an all-ones matmul accumulated alongside PV.
"""

import math

import numpy as np

try:
    import concourse.bacc as bacc  # noqa: F401
except ImportError:
    import sys

    for _p in ("/root/.axon_site/_ro/trn_rl_repo", "/opt/trn_rl_repo"):
        if _p not in sys.path:
            sys.path.insert(0, _p)

import concourse.bacc as bacc
import concourse.mybir as mybir
import concourse.tile as tile
from concourse.bass_utils import run_bass_kernel_spmd

# model dims
H, DN, DR, DV = 32, 128, 64, 128
HID, QR, KVR = 4096, 1536, 512
EPS, MAXP = 1e-6, 4096
B, S = 2, 2048
P = 128
TT = 512  # token tile (matmul moving dim)
NH = 8  # heads per core
NCORES = 8
SCALE = 1.0 / math.sqrt(DN + DR)
HIDC = HID // P  # 32
QRC = QR // P  # 12
KVRC = KVR // P  # 4
NLAT = QRC + KVRC  # 16 gathered latent chunks

F32 = mybir.dt.float32
BF16 = mybir.dt.bfloat16
NP_BF16 = mybir.dt.np(BF16)

EXP_FN = mybir.ActivationFunctionType.Exp
SQRT_FN = mybir.ActivationFunctionType.Sqrt

SEQPAR = True  # sequence-parallel a-projections via AllGather


def build_nc(tb=S, seqpar=SEQPAR):
    """Build the per-core Bass program (same program on all 8 cores)."""
    import os as _os
    phmax = int(_os.environ.get("PHMAX", "9"))
    ntt = tb // TT  # token tiles for phases B..E
    ntc = tb // P  # token chunks
    tta = tb // 4  # a-projection slice length per core
    nc = bacc.Bacc("TRN2", target_bir_lowering=False, debug=False,
                   num_devices=NCORES)

    xT = nc.dram_tensor("xT", [HID, tb], BF16, kind="ExternalInput")
    qa_wT = nc.dram_tensor("qa_wT", [HID, QR], BF16, kind="ExternalInput")
    kva_wT = nc.dram_tensor("kva_wT", [HID, KVR], BF16, kind="ExternalInput")
    kr_wT = nc.dram_tensor("kr_wT", [HID, NH * DR], BF16, kind="ExternalInput")
    qb_wT = nc.dram_tensor("qb_wT", [QR, NH * (DN + DR)], BF16,
                           kind="ExternalInput")
    kvbk_wT = nc.dram_tensor("kvbk_wT", [KVR, NH * DN], BF16,
                             kind="ExternalInput")
    kvbv_wT = nc.dram_tensor("kvbv_wT", [KVR, NH * DV], BF16,
                             kind="ExternalInput")
    o_wT = nc.dram_tensor("o_wT", [NH * DV, HID], BF16, kind="ExternalInput")
    cos_in = nc.dram_tensor("cos_rep", [P, tb], F32, kind="ExternalInput")
    sin_in = nc.dram_tensor("sin_rep", [P, tb], F32, kind="ExternalInput")
    if seqpar:
        xA = nc.dram_tensor("xA", [HID, tta], BF16, kind="ExternalInput")
    out_part = nc.dram_tensor("out_part", [tb, HID], F32, kind="ExternalOutput")

    x_ap = xT[:, :].rearrange("(c p) t -> p c t", p=P)
    qa_ap = qa_wT[:, :].rearrange("(c p) m -> p c m", p=P)
    kva_ap = kva_wT[:, :].rearrange("(c p) m -> p c m", p=P)
    kr_ap = kr_wT[:, :].rearrange("(c p) m -> p c m", p=P)
    qb_ap = qb_wT[:, :].rearrange("(c p) m -> p c m", p=P)
    kvbk_ap = kvbk_wT[:, :].rearrange("(c p) m -> p c m", p=P)
    kvbv_ap = kvbv_wT[:, :].rearrange("(c p) m -> p c m", p=P)
    ow_ap = o_wT[:, :].rearrange("(c p) m -> p c m", p=P)
    if seqpar:
        xa_ap = xA[:, :].rearrange("(c p) t -> p c t", p=P)

    with tile.TileContext(nc) as tc:
        with tc.tile_pool(name="const", bufs=1) as constp, \
             tc.tile_pool(name="dram", bufs=1, space="DRAM") as dram:
            ones_f = constp.tile([P, P], F32)
            nc.any.memset(ones_f[:], 1.0)
            ones_b = constp.tile([P, P], BF16)
            nc.vector.tensor_copy(out=ones_b[:], in_=ones_f[:])
            eps_sb = constp.tile([P, 1], F32)
            nc.any.memset(eps_sb[:], EPS)
            cos_sb = constp.tile([P, tb], F32)
            sin_sb = constp.tile([P, tb], F32)
            nc.sync.dma_start(out=cos_sb[:], in_=cos_in[:, :])
            nc.sync.dma_start(out=sin_sb[:], in_=sin_in[:, :])

            # gathered normalized latents: block g = tokens [g*tta,(g+1)*tta)
            # split q/kv so the small kv gather fires early and kv_b starts
            # while the q latents are still being computed/gathered
            latq_in = dram.tile([QRC, P, tta], BF16)
            latq_all = dram.tile([4 * QRC, P, tta], BF16)
            latkv_in = dram.tile([KVRC, P, tta], BF16)
            latkv_all = dram.tile([4 * KVRC, P, tta], BF16)
            qnope_d = dram.tile([P, NH, tb], BF16)
            qrope_d = dram.tile([P, NH * DR // P, tb], BF16)
            knope_d = dram.tile([P, NH, tb], BF16)
            krope_d = dram.tile([P, NH * DR // P, tb], BF16)
            v_d = dram.tile([P, ntc, NH * DV], BF16)
            attn_d = dram.tile([P, NH, tb], BF16)

            def rope_evict(lo_src, hi_src, tsl, pool, tag):
                """lo/hi chunk pair [P, n] (4 heads x 32 rows) -> rotate."""
                t1 = pool.tile([P, TT], F32, tag=tag, name="rt1")
                t2 = pool.tile([P, TT], F32, tag=tag, name="rt2")
                n = tsl.stop - tsl.start
                nc.vector.tensor_mul(out=t1[:, :n], in0=lo_src[:],
                                     in1=cos_sb[:, tsl])
                nc.vector.tensor_mul(out=t2[:, :n], in0=hi_src[:],
                                     in1=sin_sb[:, tsl])
                lo_o = pool.tile([P, TT], BF16, tag=tag, name="rlo")
                nc.vector.tensor_sub(out=lo_o[:, :n], in0=t1[:, :n],
                                     in1=t2[:, :n])
                t3 = pool.tile([P, TT], F32, tag=tag, name="rt3")
                t4 = pool.tile([P, TT], F32, tag=tag, name="rt4")
                nc.vector.tensor_mul(out=t3[:, :n], in0=hi_src[:],
                                     in1=cos_sb[:, tsl])
                nc.vector.tensor_mul(out=t4[:, :n], in0=lo_src[:],
                                     in1=sin_sb[:, tsl])
                hi_o = pool.tile([P, TT], BF16, tag=tag, name="rhi")
                nc.vector.tensor_add(out=hi_o[:, :n], in0=t3[:, :n],
                                     in1=t4[:, :n])
                return lo_o, hi_o

            def rope_store(lo_o, hi_o, ci, dst_d, tsl):
                n = tsl.stop - tsl.start
                for hh in range(4):
                    h = ci * 4 + hh
                    dc, dp = h // 2, 64 * (h % 2)
                    nc.sync.dma_start(
                        out=dst_d[dp:dp + 32, dc, tsl],
                        in_=lo_o[32 * hh:32 * hh + 32, :n])
                    nc.sync.dma_start(
                        out=dst_d[dp + 32:dp + 64, dc, tsl],
                        in_=hi_o[32 * hh:32 * hh + 32, :n])

            # ------- Phase A1: seq-sliced q_a/kv_a + rms-norm + gather -------
            # groups of output chunks; contraction over HID (32 k-chunks)
            with tc.tile_pool(name="apw", bufs=7) as wpool, \
                 tc.tile_pool(name="apx", bufs=2) as xpool, \
                 tc.tile_pool(name="apraw", bufs=NLAT + 1) as rawp, \
                 tc.tile_pool(name="apev", bufs=8) as evp, \
                 tc.tile_pool(name="apacc", bufs=6, space="PSUM") as accp, \
                 tc.tile_pool(name="apstat", bufs=2, space="PSUM") as statp:
                slices = range(1) if seqpar else range(4)
                if phmax < 1:
                    slices = range(0)
                def norm_and_ship(kind, raws, stats, g4):
                    rank = QR if kind == "q" else KVR
                    nm = QRC if kind == "q" else KVRC
                    l_in = latq_in if kind == "q" else latkv_in
                    l_all = latq_all if kind == "q" else latkv_all
                    sdev = evp.tile([P, tta], F32, tag="ev")
                    nc.scalar.activation(sdev[:], stats[kind][:], SQRT_FN,
                                         bias=eps_sb[:], scale=1.0 / rank)
                    rstd = evp.tile([P, tta], F32, tag="ev")
                    nc.vector.reciprocal(rstd[:], sdev[:])
                    for m in range(nm):
                        nrm = evp.tile([P, tta], BF16, tag="ev")
                        nc.vector.tensor_mul(out=nrm[:],
                                             in0=raws[(kind, m)][:],
                                             in1=rstd[:])
                        if seqpar:
                            nc.sync.dma_start(out=l_in[m], in_=nrm[:])
                        else:
                            nc.sync.dma_start(out=l_all[g4 * nm + m],
                                              in_=nrm[:])

                def gather(kind):
                    l_in = latq_in if kind == "q" else latkv_in
                    l_all = latq_all if kind == "q" else latkv_all
                    nc.gpsimd.collective_compute(
                        "AllGather",
                        mybir.AluOpType.bypass,
                        replica_groups=[[0, 1, 2, 3], [4, 5, 6, 7]],
                        ins=[l_in.opt()],
                        outs=[l_all.opt()],
                    )

                for g4 in slices:
                    asl = slice(g4 * tta, (g4 + 1) * tta)
                    xs = xpool.tile([P, HIDC, tta], BF16, tag="apx",
                                    name=f"xs{g4}")
                    if seqpar:
                        nc.sync.dma_start(out=xs[:], in_=xa_ap[:, :, :])
                    else:
                        nc.sync.dma_start(out=xs[:], in_=x_ap[:, :, asl])
                    raws = {}
                    stats = {}
                    for kind, m0, m1 in [("kv", 0, 4), ("q", 0, 6),
                                         ("q", 6, 12)]:
                        src = qa_ap if kind == "q" else kva_ap
                        wts = []
                        for m in range(m0, m1):
                            wt = wpool.tile([P, HIDC, P], BF16, tag="apw",
                                            name=f"apw{kind}{g4}_{m}")
                            nc.sync.dma_start(out=wt[:],
                                              in_=src[:, :, m * P:(m + 1) * P])
                            wts.append(wt)
                        accs = [accp.tile([P, tta], F32, tag="acc",
                                          name=f"acc{kind}{g4}_{m}")
                                for m in range(m0, m1)]
                        for k in range(HIDC):
                            for mi in range(m1 - m0):
                                nc.tensor.matmul(
                                    accs[mi][:], wts[mi][:, k, :], xs[:, k, :],
                                    start=(k == 0), stop=(k == HIDC - 1))
                        if kind not in stats:
                            stats[kind] = statp.tile(
                                [P, tta], F32, tag="stat",
                                name=f"stat{kind}_{g4}")
                        stat = stats[kind]
                        nm = QRC if kind == "q" else KVRC
                        for mi, m in enumerate(range(m0, m1)):
                            raw = rawp.tile([P, tta], BF16, tag="raw",
                                            name=f"raw{kind}{g4}_{m}")
                            nc.vector.tensor_copy(out=raw[:], in_=accs[mi][:])
                            raws[(kind, m)] = raw
                            sq = evp.tile([P, tta], BF16, tag="ev")
                            nc.vector.tensor_mul(out=sq[:], in0=raw[:],
                                                 in1=raw[:])
                            nc.tensor.matmul(stat[:], ones_b[:], sq[:],
                                             start=(m == 0), stop=(m == nm - 1))
                        if kind == "kv":
                            # kv latents are done: normalize + gather them now
                            norm_and_ship("kv", raws, stats, g4)
                            if seqpar and phmax >= 1 and g4 == slices[-1]:
                                gather("kv")
                    norm_and_ship("q", raws, stats, g4)
                    if seqpar and phmax >= 1 and g4 == slices[-1]:
                        gather("q")

            # ------- Phase A2: k_rope (all tb tokens, this core's heads) -----
            with tc.tile_pool(name="krw", bufs=4) as wpool, \
                 tc.tile_pool(name="krx", bufs=4) as xpool, \
                 tc.tile_pool(name="krev", bufs=10) as evp, \
                 tc.tile_pool(name="kracc", bufs=5, space="PSUM") as accp:
                wts = []
                for m in range(4):
                    wt = wpool.tile([P, HIDC, P], BF16, tag="krw",
                                    name=f"krw{m}")
                    nc.sync.dma_start(out=wt[:], in_=kr_ap[:, :, m * P:(m + 1) * P])
                    wts.append(wt)
                for t in range(ntt if phmax >= 2 else 0):
                    tsl = slice(t * TT, (t + 1) * TT)
                    accs = [accp.tile([P, TT], F32, tag="acc",
                                      name=f"kracc{t}_{m}") for m in range(4)]
                    for k in range(HIDC):
                        xt = xpool.tile([P, TT], BF16, tag="krx",
                                        name=f"krx{t}_{k}")
                        nc.sync.dma_start(out=xt[:], in_=x_ap[:, k, tsl])
                        for mi in range(4):
                            nc.tensor.matmul(
                                accs[mi][:], wts[mi][:, k, :], xt[:],
                                start=(k == 0), stop=(k == HIDC - 1))
                    # chunks [lo0, lo1, hi0, hi1] -> rope
                    for ci in range(2):
                        lo_o, hi_o = rope_evict(accs[ci], accs[2 + ci], tsl,
                                                evp, "ev")
                        rope_store(lo_o, hi_o, ci, krope_d, tsl)

            def load_lat(pool, tag, name, koff, nk, t):
                """Load latent chunks [koff, koff+nk) for token tile t."""
                tiles = []
                src_all = {"q": latq_all, "kv": latkv_all}
                src_nm = {"q": QRC, "kv": KVRC}
                for kk in range(nk):
                    qn = pool.tile([P, TT], BF16, tag=tag,
                                   name=f"{name}{t}_{kk}")
                    for bk in range(4):
                        lo, hi = bk * tta, (bk + 1) * tta
                        if lo >= t * TT and hi <= (t + 1) * TT:
                            nc.sync.dma_start(
                                out=qn[:, lo - t * TT:hi - t * TT],
                                in_=src_all[koff][bk * src_nm[koff] + kk])
                    tiles.append(qn)
                return tiles

            # ------- Phase C: kv_b (k_nope + v) -----------------------------
            # emitted before q_b: it only needs the small kv gather
            with tc.tile_pool(name="kvw", bufs=1) as kvwp, \
                 tc.tile_pool(name="kvn", bufs=KVRC + 1) as kvnp, \
                 tc.tile_pool(name="kvev", bufs=4) as evp, \
                 tc.tile_pool(name="kvps", bufs=4, space="PSUM") as kvps:
                kbw = kvwp.tile([P, KVRC, NH * DN], BF16)
                vbw = kvwp.tile([P, KVRC, NH * DV], BF16)
                nc.sync.dma_start(out=kbw[:], in_=kvbk_ap[:, :, :])
                nc.sync.dma_start(out=vbw[:], in_=kvbv_ap[:, :, :])
                for t in range(ntt if phmax >= 4 else 0):
                    tsl = slice(t * TT, (t + 1) * TT)
                    kvn = load_lat(kvnp, "kvn", "kvn", "kv", KVRC, t)
                    for m in range(NH):
                        ps = kvps.tile([P, TT], F32, tag="kps",
                                       name=f"kb{t}_{m}")
                        for k in range(KVRC):
                            nc.tensor.matmul(ps[:], kbw[:, k, m * P:(m + 1) * P],
                                             kvn[k][:], start=(k == 0),
                                             stop=(k == KVRC - 1))
                        o = evp.tile([P, TT], BF16, tag="ev")
                        nc.vector.tensor_copy(out=o[:], in_=ps[:])
                        nc.sync.dma_start(out=knope_d[:, m, tsl], in_=o[:])
                    for tc8 in range(TT // P):
                        for vc in range(NH * DV // TT):
                            ps = kvps.tile([P, TT], F32, tag="vps",
                                           name=f"v{t}_{tc8}_{vc}")
                            for k in range(KVRC):
                                nc.tensor.matmul(
                                    ps[:],
                                    kvn[k][:, tc8 * P:(tc8 + 1) * P],
                                    vbw[:, k, vc * TT:(vc + 1) * TT],
                                    start=(k == 0), stop=(k == KVRC - 1))
                            o = evp.tile([P, TT], BF16, tag="ev")
                            nc.vector.tensor_copy(out=o[:], in_=ps[:])
                            nc.sync.dma_start(
                                out=v_d[:, t * (TT // P) + tc8,
                                        vc * TT:(vc + 1) * TT],
                                in_=o[:])

            # ------- Phase B: q_b + q rope ----------------------------------
            with tc.tile_pool(name="qbw", bufs=1) as qbwp, \
                 tc.tile_pool(name="qbn", bufs=QRC + 1) as qnp, \
                 tc.tile_pool(name="qbev", bufs=10) as evp, \
                 tc.tile_pool(name="qbps", bufs=8, space="PSUM") as qbps:
                qbw = qbwp.tile([P, QRC, NH * (DN + DR)], BF16)
                nc.sync.dma_start(out=qbw[:], in_=qb_ap[:, :, :])
                for t in range(ntt if phmax >= 3 else 0):
                    tsl = slice(t * TT, (t + 1) * TT)
                    qn = load_lat(qnp, "qn", "qn", "q", QRC, t)
                    rope_ps = {}
                    for m in range(QRC):
                        ps = qbps.tile([P, TT], F32, tag="qbps",
                                       name=f"qb{t}_{m}")
                        for k in range(QRC):
                            nc.tensor.matmul(ps[:], qbw[:, k, m * P:(m + 1) * P],
                                             qn[k][:], start=(k == 0),
                                             stop=(k == QRC - 1))
                        if m < NH:
                            o = evp.tile([P, TT], BF16, tag="ev")
                            nc.vector.tensor_copy(out=o[:], in_=ps[:])
                            nc.sync.dma_start(out=qnope_d[:, m, tsl], in_=o[:])
                        else:
                            rope_ps[m - NH] = ps
                    for ci in range(2):
                        lo_o, hi_o = rope_evict(rope_ps[ci], rope_ps[2 + ci],
                                                tsl, evp, "ev")
                        rope_store(lo_o, hi_o, ci, qrope_d, tsl)

            # ------- Phase D+E: attention + o_proj --------------------------
            # o_proj token groups are emitted inside the LAST head pair's
            # query loop: E's matmuls fill the PE stalls of D's ACT-bound
            # stretches, and E's tail shrinks to one token group.
            nkt = tb // P  # key tiles
            with tc.tile_pool(name="ath", bufs=2) as hp, \
                 tc.tile_pool(name="atex", bufs=6) as exp_p, \
                 tc.tile_pool(name="atev", bufs=4) as evp, \
                 tc.tile_pool(name="oww", bufs=1) as owp, \
                 tc.tile_pool(name="oin", bufs=3) as inp, \
                 tc.tile_pool(name="oev", bufs=3) as oevp, \
                 tc.tile_pool(name="atsc", bufs=3, space="PSUM") as scp, \
                 tc.tile_pool(name="atpv", bufs=1, space="PSUM") as pvp, \
                 tc.tile_pool(name="atden", bufs=1, space="PSUM") as denp, \
                 tc.tile_pool(name="ops", bufs=2, space="PSUM") as ops:
                oww = owp.tile([P, NH * DV // P, HID], BF16)
                nc.sync.dma_start(out=oww[:], in_=ow_ap[:, :, :])

                def emit_e(t8):
                    at = inp.tile([P, NH, P], BF16, tag="at", name=f"at{t8}")
                    nc.sync.dma_start(out=at[:],
                                      in_=attn_d[:, :, t8 * P:(t8 + 1) * P])
                    for n in range(HID // TT):
                        ps = ops.tile([P, TT], F32, tag="ops",
                                      name=f"o{t8}_{n}")
                        for k in range(NH * DV // P):
                            nc.tensor.matmul(ps[:], at[:, k, :],
                                             oww[:, k, n * TT:(n + 1) * TT],
                                             start=(k == 0),
                                             stop=(k == NH * DV // P - 1))
                        o = oevp.tile([P, TT], F32, tag="ev")
                        nc.vector.tensor_copy(out=o[:], in_=ps[:])
                        nc.sync.dma_start(
                            out=out_part[t8 * P:(t8 + 1) * P,
                                         n * TT:(n + 1) * TT],
                            in_=o[:])

                for j in range(NH // 2 if phmax >= 5 else 0):
                    h0, h1 = 2 * j, 2 * j + 1
                    kn0 = hp.tile([P, tb], BF16, tag="kn0", name=f"kn0_{j}")
                    kn1 = hp.tile([P, tb], BF16, tag="kn1", name=f"kn1_{j}")
                    nc.sync.dma_start(out=kn0[:], in_=knope_d[:, h0, :])
                    nc.sync.dma_start(out=kn1[:], in_=knope_d[:, h1, :])
                    krj = hp.tile([P, tb], BF16, tag="krj", name=f"krj{j}")
                    nc.sync.dma_start(out=krj[:], in_=krope_d[:, j, :])
                    v0 = hp.tile([P, nkt, DV], BF16, tag="v0", name=f"v0_{j}")
                    v1 = hp.tile([P, nkt, DV], BF16, tag="v1", name=f"v1_{j}")
                    nc.sync.dma_start(out=v0[:], in_=v_d[:, :, h0 * DV:(h0 + 1) * DV])
                    nc.sync.dma_start(out=v1[:], in_=v_d[:, :, h1 * DV:(h1 + 1) * DV])
                    qn0 = hp.tile([P, tb], BF16, tag="qn0", name=f"qn0_{j}")
                    qn1 = hp.tile([P, tb], BF16, tag="qn1", name=f"qn1_{j}")
                    nc.sync.dma_start(out=qn0[:], in_=qnope_d[:, h0, :])
                    nc.sync.dma_start(out=qn1[:], in_=qnope_d[:, h1, :])
                    qrj = hp.tile([P, tb], BF16, tag="qrj", name=f"qrj{j}")
                    nc.sync.dma_start(out=qrj[:], in_=qrope_d[:, j, :])
                    for qt in range(ntt):
                        qsl = slice(qt * TT, (qt + 1) * TT)
                        pv0 = pvp.tile([P, TT], F32, tag="pv0",
                                       name=f"pv0_{j}_{qt}")
                        pv1 = pvp.tile([P, TT], F32, tag="pv1",
                                       name=f"pv1_{j}_{qt}")
                        # pair-packed denominators: h0 -> rows 0:64, h1 ->
                        # rows 64:128 (concurrent M=64 col-tiled matmuls)
                        denw = denp.tile([P, TT], F32, tag="den",
                                         name=f"den_{j}_{qt}")
                        for kt in range(nkt):
                            ksl = slice(kt * P, (kt + 1) * P)
                            sc0 = scp.tile([P, TT], F32, tag="sc",
                                           name=f"sc0_{j}_{qt}_{kt}")
                            sc1 = scp.tile([P, TT], F32, tag="sc",
                                           name=f"sc1_{j}_{qt}_{kt}")
                            nc.tensor.matmul(sc0[:], kn0[:, ksl], qn0[:, qsl],
                                             start=True, stop=False)
                            nc.tensor.matmul(sc1[:], kn1[:, ksl], qn1[:, qsl],
                                             start=True, stop=False)
                            # K=64 rope matmuls: disjoint row groups run
                            # concurrently in the PE array (tile_position)
                            nc.tensor.matmul(sc0[:], krj[0:64, ksl],
                                             qrj[0:64, qsl],
                                             start=False, stop=True,
                                             tile_position=(0, 0))
                            nc.tensor.matmul(sc1[:], krj[64:128, ksl],
                                             qrj[64:128, qsl],
                                             start=False, stop=True,
                                             tile_position=(64, 0))
                            ex0 = exp_p.tile([P, TT], BF16, tag="ex",
                                             name=f"ex0_{j}_{qt}_{kt}")
                            ex1 = exp_p.tile([P, TT], BF16, tag="ex",
                                             name=f"ex1_{j}_{qt}_{kt}")
                            nc.scalar.activation(ex0[:], sc0[:], EXP_FN,
                                                 scale=SCALE)
                            nc.scalar.activation(ex1[:], sc1[:], EXP_FN,
                                                 scale=SCALE)
                            st, sp = kt == 0, kt == nkt - 1
                            nc.tensor.matmul(pv0[:], v0[:, kt, :], ex0[:],
                                             start=st, stop=sp)
                            nc.tensor.matmul(pv1[:], v1[:, kt, :], ex1[:],
                                             start=st, stop=sp)
                            nc.tensor.matmul(denw[0:64, :], ones_b[:, 0:64],
                                             ex0[:], start=st, stop=sp,
                                             tile_position=(0, 0),
                                             skip_group_check=True)
                            nc.tensor.matmul(denw[64:128, :], ones_b[:, 0:64],
                                             ex1[:], start=st, stop=sp,
                                             tile_position=(0, 64),
                                             skip_group_check=True)
                        recipw = evp.tile([P, TT], F32, tag="evr",
                                          name=f"recipw_{j}_{qt}")
                        nc.vector.reciprocal(recipw[:], denw[:])
                        for h, pv, r0 in ((h0, pv0, 0), (h1, pv1, 64)):
                            rfull = evp.tile([P, TT], F32, tag="evr",
                                             name=f"rfull_{h}_{qt}")
                            nc.sync.dma_start(out=rfull[0:64, :],
                                              in_=recipw[r0:r0 + 64, :])
                            nc.sync.dma_start(out=rfull[64:128, :],
                                              in_=recipw[r0:r0 + 64, :])
                            ao = evp.tile([P, TT], BF16, tag="ev", name="ao")
                            nc.vector.tensor_mul(out=ao[:], in0=pv[:],
                                                 in1=rfull[:])
                            nc.sync.dma_start(out=attn_d[:, h, qsl], in_=ao[:])

            # ------- Phase E: o_proj (partial) ------------------------------
            with tc.tile_pool(name="oww", bufs=1) as owp, \
                 tc.tile_pool(name="oin", bufs=3) as inp, \
                 tc.tile_pool(name="oev", bufs=3) as evp, \
                 tc.tile_pool(name="ops", bufs=8, space="PSUM") as ops:
                oww = owp.tile([P, NH * DV // P, HID], BF16)
                nc.sync.dma_start(out=oww[:], in_=ow_ap[:, :, :])
                for t8 in range(ntc if phmax >= 6 else 0):
                    at = inp.tile([P, NH, P], BF16, tag="at", name=f"at{t8}")
                    nc.sync.dma_start(out=at[:],
                                      in_=attn_d[:, :, t8 * P:(t8 + 1) * P])
                    for n in range(HID // TT):
                        ps = ops.tile([P, TT], F32, tag="ops", name=f"o{t8}_{n}")
                        for k in range(NH * DV // P):
                            nc.tensor.matmul(ps[:], at[:, k, :],
                                             oww[:, k, n * TT:(n + 1) * TT],
                                             start=(k == 0),
                                             stop=(k == NH * DV // P - 1))
                        o = evp.tile([P, TT], F32, tag="ev")
                        nc.vector.tensor_copy(out=o[:], in_=ps[:])
                        nc.sync.dma_start(
                            out=out_part[t8 * P:(t8 + 1) * P,
                                         n * TT:(n + 1) * TT],
                            in_=o[:])

    nc.compile()
    return nc


# ---------------------------------------------------------------------------
# host-side packing
# ---------------------------------------------------------------------------

def _rope_tables():
    inv_freq = 1.0 / (10000.0 ** (np.arange(0, DR, 2, dtype=np.float32) / DR))
    t = np.arange(MAXP, dtype=np.float32)
    freqs = np.outer(t, inv_freq)
    emb = np.concatenate([freqs, freqs], axis=-1)
    return np.cos(emb).astype(np.float32), np.sin(emb).astype(np.float32)


def core_weights(g, q_a_w, q_a_ln_w, q_b_w, kv_a_w, kv_a_ln_w, kv_b_w,
                 k_rope_w, o_w):
    """Pack the weight set for head-group g (heads g*8 .. g*8+8)."""
    heads = range(g * NH, (g + 1) * NH)
    qb_eff = (q_b_w * q_a_ln_w[None, :]).astype(np.float32)
    kvb_eff = (kv_b_w * kv_a_ln_w[None, :]).astype(np.float32)

    nope_rows = np.concatenate(
        [np.arange(h * (DN + DR), h * (DN + DR) + DN) for h in heads])
    lo_rows = np.concatenate(
        [np.arange(h * (DN + DR) + DN, h * (DN + DR) + DN + 32) for h in heads])
    hi_rows = np.concatenate(
        [np.arange(h * (DN + DR) + DN + 32, h * (DN + DR) + DN + 64)
         for h in heads])
    qb_rows = np.concatenate([nope_rows, lo_rows, hi_rows])

    k_rows = np.concatenate(
        [np.arange(h * (DN + DV), h * (DN + DV) + DN) for h in heads])
    v_rows = np.concatenate(
        [np.arange(h * (DN + DV) + DN, (h + 1) * (DN + DV)) for h in heads])

    kr_lo = np.concatenate([np.arange(h * DR, h * DR + 32) for h in heads])
    kr_hi = np.concatenate([np.arange(h * DR + 32, (h + 1) * DR) for h in heads])
    kr_rows = np.concatenate([kr_lo, kr_hi])

    o_cols = np.concatenate([np.arange(h * DV, (h + 1) * DV) for h in heads])

    def c(a):
        return np.ascontiguousarray(a).astype(NP_BF16)

    return {
        "qa_wT": c(q_a_w.T),
        "kva_wT": c(kv_a_w.T),
        "kr_wT": c(k_rope_w[kr_rows].T),
        "qb_wT": c(qb_eff[qb_rows].T),
        "kvbk_wT": c(kvb_eff[k_rows].T),
        "kvbv_wT": c(kvb_eff[v_rows].T),
        "o_wT": c(o_w[:, o_cols].T),
    }


def core_inputs(c, hidden_states, position_ids, weight_sets, tb=S,
                seqpar=SEQPAR):
    b, g = c // 4, c % 4
    tta = tb // 4
    cos_tab, sin_tab = _rope_tables()
    pos = np.asarray(position_ids[b][:tb])
    cos_b = cos_tab[pos]  # [tb, DR]
    sin_b = sin_tab[pos]
    cos_rep = np.ascontiguousarray(np.tile(cos_b[:, :32].T, (4, 1)))
    sin_rep = np.ascontiguousarray(np.tile(sin_b[:, :32].T, (4, 1)))
    x = np.asarray(hidden_states[b][:tb], dtype=np.float32)
    xTb = np.ascontiguousarray(x.T).astype(NP_BF16)
    im = {"xT": xTb, "cos_rep": cos_rep, "sin_rep": sin_rep}
    if seqpar:
        im["xA"] = np.ascontiguousarray(xTb[:, g * tta:(g + 1) * tta])
    im.update(weight_sets[g])
    return im


_CACHE = {}


def _get_nc(tb=S):
    if tb not in _CACHE:
        _CACHE[tb] = build_nc(tb)
    return _CACHE[tb]


def kernel(hidden_states, position_ids, q_a_w, q_a_ln_w, q_b_w,
           kv_a_w, kv_a_ln_w, kv_b_w, k_rope_w, o_w):
    hidden_states = np.asarray(hidden_states, dtype=np.float32)
    weight_sets = [
        core_weights(g, np.asarray(q_a_w, np.float32),
                     np.asarray(q_a_ln_w, np.float32),
                     np.asarray(q_b_w, np.float32),
                     np.asarray(kv_a_w, np.float32),
                     np.asarray(kv_a_ln_w, np.float32),
                     np.asarray(kv_b_w, np.float32),
                     np.asarray(k_rope_w, np.float32),
                     np.asarray(o_w, np.float32))
        for g in range(4)
    ]
    in_maps = [core_inputs(c, hidden_states, position_ids, weight_sets)
               for c in range(NCORES)]
    nc = _get_nc()
    res = run_bass_kernel_spmd(nc, in_maps, core_ids=list(range(NCORES)))
    out = np.zeros((B, S, HID), dtype=np.float64)
    for c in range(NCORES):
        out[c // 4] += res.results[c]["out_part"]
    return out.astype(np.float32)


# revision 29
# speedup vs baseline: 1.1333x; 1.1333x over previous
"""DeepSeekV3 MLA attention kernel for Trainium2 (8 NeuronCores, Bass/Tile).

Sharding: core c -> batch b = c // 4, head-group g = c % 4 (8 of 32 heads).
Each core runs the layer for its batch restricted to its heads and emits a
partial o_proj output [2048, 4096]; the host sums the 4 partials per batch.

The shared a-projections (q_a, kv_a) are *sequence-parallel*: each core
computes and RMS-normalizes the latents for its 512-token slice only, then an
AllGather over the 4-core batch group replicates the full normalized latents
(2.1 MB/rank, bf16) while the per-head k_rope projection keeps the PE busy.
This removes the 4x replicated a-projection compute of the naive layout
(~330 us of PE time per core).

Layouts (feature-major, [128, chunks, tokens]); all matmul operands are bf16
(full PE rate, half the DMA/SBUF of fp32r), PSUM accumulation fp32:
  - x is fed transposed (xT [4096, 2048]); matmuls contract over the
    partition dim with N = 512 token tiles (one PSUM bank).
  - RoPE halves are packed [4*lo(128) | 4*hi(128)] per 4 heads so the rotate
    is partition-aligned full-lane DVE work; a DMA rearrange then stores
    per-head-contiguous [64] blocks that attention contracts over (K=64).
    Head pairs (2j, 2j+1) land in partition halves [0:64], [64:128] of the
    same chunk, so attention row-packs the two K=64 rope matmuls into one
    PE pass via tile_position (concurrent sub-array execution).
  - Softmax skips the max-subtraction (scores are O(5), exp safe in fp32);
    denominators come from an all-ones matmul accumulated alongside PV.
"""

import math

import numpy as np

try:
    import concourse.bacc as bacc  # noqa: F401
except ImportError:
    import sys

    for _p in ("/root/.axon_site/_ro/trn_rl_repo", "/opt/trn_rl_repo"):
        if _p not in sys.path:
            sys.path.insert(0, _p)

import concourse.bacc as bacc
import concourse.mybir as mybir
import concourse.tile as tile
from concourse.bass_utils import run_bass_kernel_spmd

# model dims
H, DN, DR, DV = 32, 128, 64, 128
HID, QR, KVR = 4096, 1536, 512
EPS, MAXP = 1e-6, 4096
B, S = 2, 2048
P = 128
TT = 512  # token tile (matmul moving dim)
NH = 8  # heads per core
NCORES = 8
SCALE = 1.0 / math.sqrt(DN + DR)
HIDC = HID // P  # 32
QRC = QR // P  # 12
KVRC = KVR // P  # 4
NLAT = QRC + KVRC  # 16 gathered latent chunks

F32 = mybir.dt.float32
BF16 = mybir.dt.bfloat16
NP_BF16 = mybir.dt.np(BF16)

EXP_FN = mybir.ActivationFunctionType.Exp
SQRT_FN = mybir.ActivationFunctionType.Sqrt

SEQPAR = True  # sequence-parallel a-projections via AllGather


def build_nc(tb=S, seqpar=SEQPAR):
    """Build the per-core Bass program (same program on all 8 cores)."""
    import os as _os
    phmax = int(_os.environ.get("PHMAX", "9"))
    ntt = tb // TT  # token tiles for phases B..E
    ntc = tb // P  # token chunks
    tta = tb // 4  # a-projection slice length per core
    nc = bacc.Bacc("TRN2", target_bir_lowering=False, debug=False,
                   num_devices=NCORES)

    xT = nc.dram_tensor("xT", [HID, tb], BF16, kind="ExternalInput")
    qa_wT = nc.dram_tensor("qa_wT", [HID, QR], BF16, kind="ExternalInput")
    kva_wT = nc.dram_tensor("kva_wT", [HID, KVR], BF16, kind="ExternalInput")
    kr_wT = nc.dram_tensor("kr_wT", [HID, NH * DR], BF16, kind="ExternalInput")
    qb_wT = nc.dram_tensor("qb_wT", [QR, NH * (DN + DR)], BF16,
                           kind="ExternalInput")
    kvbk_wT = nc.dram_tensor("kvbk_wT", [KVR, NH * DN], BF16,
                             kind="ExternalInput")
    kvbv_wT = nc.dram_tensor("kvbv_wT", [KVR, NH * DV], BF16,
                             kind="ExternalInput")
    o_wT = nc.dram_tensor("o_wT", [NH * DV, HID], BF16, kind="ExternalInput")
    cos_in = nc.dram_tensor("cos_rep", [P, tb], F32, kind="ExternalInput")
    sin_in = nc.dram_tensor("sin_rep", [P, tb], F32, kind="ExternalInput")
    if seqpar:
        xA = nc.dram_tensor("xA", [HID, tta], BF16, kind="ExternalInput")
    out_part = nc.dram_tensor("out_part", [tb, HID], F32, kind="ExternalOutput")

    x_ap = xT[:, :].rearrange("(c p) t -> p c t", p=P)
    qa_ap = qa_wT[:, :].rearrange("(c p) m -> p c m", p=P)
    kva_ap = kva_wT[:, :].rearrange("(c p) m -> p c m", p=P)
    kr_ap = kr_wT[:, :].rearrange("(c p) m -> p c m", p=P)
    qb_ap = qb_wT[:, :].rearrange("(c p) m -> p c m", p=P)
    kvbk_ap = kvbk_wT[:, :].rearrange("(c p) m -> p c m", p=P)
    kvbv_ap = kvbv_wT[:, :].rearrange("(c p) m -> p c m", p=P)
    ow_ap = o_wT[:, :].rearrange("(c p) m -> p c m", p=P)
    if seqpar:
        xa_ap = xA[:, :].rearrange("(c p) t -> p c t", p=P)

    with tile.TileContext(nc) as tc:
        with tc.tile_pool(name="const", bufs=1) as constp, \
             tc.tile_pool(name="dram", bufs=1, space="DRAM") as dram:
            ones_f = constp.tile([P, P], F32)
            nc.any.memset(ones_f[:], 1.0)
            ones_b = constp.tile([P, P], BF16)
            nc.vector.tensor_copy(out=ones_b[:], in_=ones_f[:])
            eps_sb = constp.tile([P, 1], F32)
            nc.any.memset(eps_sb[:], EPS)
            cos_sb = constp.tile([P, tb], F32)
            sin_sb = constp.tile([P, tb], F32)
            nc.sync.dma_start(out=cos_sb[:], in_=cos_in[:, :])
            nc.sync.dma_start(out=sin_sb[:], in_=sin_in[:, :])

            # gathered normalized latents: block g = tokens [g*tta,(g+1)*tta)
            # split q/kv so the small kv gather fires early and kv_b starts
            # while the q latents are still being computed/gathered
            latq_in = dram.tile([QRC, P, tta], BF16)
            latq_all = dram.tile([4 * QRC, P, tta], BF16)
            latkv_in = dram.tile([KVRC, P, tta], BF16)
            latkv_all = dram.tile([4 * KVRC, P, tta], BF16)
            qnope_d = dram.tile([P, NH, tb], BF16)
            qrope_d = dram.tile([P, NH * DR // P, tb], BF16)
            knope_d = dram.tile([P, NH, tb], BF16)
            krope_d = dram.tile([P, NH * DR // P, tb], BF16)
            v_d = dram.tile([P, ntc, NH * DV], BF16)
            attn_d = dram.tile([P, NH, tb], BF16)

            def rope_evict(lo_src, hi_src, tsl, pool, tag):
                """lo/hi chunk pair [P, n] (4 heads x 32 rows) -> rotate."""
                t1 = pool.tile([P, TT], F32, tag=tag, name="rt1")
                t2 = pool.tile([P, TT], F32, tag=tag, name="rt2")
                n = tsl.stop - tsl.start
                nc.vector.tensor_mul(out=t1[:, :n], in0=lo_src[:],
                                     in1=cos_sb[:, tsl])
                nc.vector.tensor_mul(out=t2[:, :n], in0=hi_src[:],
                                     in1=sin_sb[:, tsl])
                lo_o = pool.tile([P, TT], BF16, tag=tag, name="rlo")
                nc.vector.tensor_sub(out=lo_o[:, :n], in0=t1[:, :n],
                                     in1=t2[:, :n])
                t3 = pool.tile([P, TT], F32, tag=tag, name="rt3")
                t4 = pool.tile([P, TT], F32, tag=tag, name="rt4")
                nc.vector.tensor_mul(out=t3[:, :n], in0=hi_src[:],
                                     in1=cos_sb[:, tsl])
                nc.vector.tensor_mul(out=t4[:, :n], in0=lo_src[:],
                                     in1=sin_sb[:, tsl])
                hi_o = pool.tile([P, TT], BF16, tag=tag, name="rhi")
                nc.vector.tensor_add(out=hi_o[:, :n], in0=t3[:, :n],
                                     in1=t4[:, :n])
                return lo_o, hi_o

            def rope_store(lo_o, hi_o, ci, dst_d, tsl):
                n = tsl.stop - tsl.start
                for hh in range(4):
                    h = ci * 4 + hh
                    dc, dp = h // 2, 64 * (h % 2)
                    nc.sync.dma_start(
                        out=dst_d[dp:dp + 32, dc, tsl],
                        in_=lo_o[32 * hh:32 * hh + 32, :n])
                    nc.sync.dma_start(
                        out=dst_d[dp + 32:dp + 64, dc, tsl],
                        in_=hi_o[32 * hh:32 * hh + 32, :n])

            # ------- Phase A1: seq-sliced q_a/kv_a + rms-norm + gather -------
            # groups of output chunks; contraction over HID (32 k-chunks)
            with tc.tile_pool(name="apw", bufs=7) as wpool, \
                 tc.tile_pool(name="apx", bufs=2) as xpool, \
                 tc.tile_pool(name="apraw", bufs=NLAT + 1) as rawp, \
                 tc.tile_pool(name="apev", bufs=8) as evp, \
                 tc.tile_pool(name="apacc", bufs=6, space="PSUM") as accp, \
                 tc.tile_pool(name="apstat", bufs=2, space="PSUM") as statp:
                slices = range(1) if seqpar else range(4)
                if phmax < 1:
                    slices = range(0)
                def norm_and_ship(kind, raws, stats, g4):
                    rank = QR if kind == "q" else KVR
                    nm = QRC if kind == "q" else KVRC
                    l_in = latq_in if kind == "q" else latkv_in
                    l_all = latq_all if kind == "q" else latkv_all
                    sdev = evp.tile([P, tta], F32, tag="ev")
                    nc.scalar.activation(sdev[:], stats[kind][:], SQRT_FN,
                                         bias=eps_sb[:], scale=1.0 / rank)
                    rstd = evp.tile([P, tta], F32, tag="ev")
                    nc.vector.reciprocal(rstd[:], sdev[:])
                    for m in range(nm):
                        nrm = evp.tile([P, tta], BF16, tag="ev")
                        nc.vector.tensor_mul(out=nrm[:],
                                             in0=raws[(kind, m)][:],
                                             in1=rstd[:])
                        if seqpar:
                            nc.sync.dma_start(out=l_in[m], in_=nrm[:])
                        else:
                            nc.sync.dma_start(out=l_all[g4 * nm + m],
                                              in_=nrm[:])

                def gather(kind):
                    l_in = latq_in if kind == "q" else latkv_in
                    l_all = latq_all if kind == "q" else latkv_all
                    nc.gpsimd.collective_compute(
                        "AllGather",
                        mybir.AluOpType.bypass,
                        replica_groups=[[0, 1, 2, 3], [4, 5, 6, 7]],
                        ins=[l_in.opt()],
                        outs=[l_all.opt()],
                    )

                for g4 in slices:
                    asl = slice(g4 * tta, (g4 + 1) * tta)
                    xs = xpool.tile([P, HIDC, tta], BF16, tag="apx",
                                    name=f"xs{g4}")
                    if seqpar:
                        nc.sync.dma_start(out=xs[:], in_=xa_ap[:, :, :])
                    else:
                        nc.sync.dma_start(out=xs[:], in_=x_ap[:, :, asl])
                    raws = {}
                    stats = {}
                    for kind, m0, m1 in [("kv", 0, 4), ("q", 0, 6),
                                         ("q", 6, 12)]:
                        src = qa_ap if kind == "q" else kva_ap
                        wts = []
                        for m in range(m0, m1):
                            wt = wpool.tile([P, HIDC, P], BF16, tag="apw",
                                            name=f"apw{kind}{g4}_{m}")
                            nc.sync.dma_start(out=wt[:],
                                              in_=src[:, :, m * P:(m + 1) * P])
                            wts.append(wt)
                        accs = [accp.tile([P, tta], F32, tag="acc",
                                          name=f"acc{kind}{g4}_{m}")
                                for m in range(m0, m1)]
                        for k in range(HIDC):
                            for mi in range(m1 - m0):
                                nc.tensor.matmul(
                                    accs[mi][:], wts[mi][:, k, :], xs[:, k, :],
                                    start=(k == 0), stop=(k == HIDC - 1))
                        if kind not in stats:
                            stats[kind] = statp.tile(
                                [P, tta], F32, tag="stat",
                                name=f"stat{kind}_{g4}")
                        stat = stats[kind]
                        nm = QRC if kind == "q" else KVRC
                        for mi, m in enumerate(range(m0, m1)):
                            raw = rawp.tile([P, tta], BF16, tag="raw",
                                            name=f"raw{kind}{g4}_{m}")
                            nc.vector.tensor_copy(out=raw[:], in_=accs[mi][:])
                            raws[(kind, m)] = raw
                            sq = evp.tile([P, tta], BF16, tag="ev")
                            nc.vector.tensor_mul(out=sq[:], in0=raw[:],
                                                 in1=raw[:])
                            nc.tensor.matmul(stat[:], ones_b[:], sq[:],
                                             start=(m == 0), stop=(m == nm - 1))
                        if kind == "kv":
                            # kv latents are done: normalize + gather them now
                            norm_and_ship("kv", raws, stats, g4)
                            if seqpar and phmax >= 1 and g4 == slices[-1]:
                                gather("kv")
                    norm_and_ship("q", raws, stats, g4)
                    if seqpar and phmax >= 1 and g4 == slices[-1]:
                        gather("q")

            # ------- Phase A2: k_rope (all tb tokens, this core's heads) -----
            with tc.tile_pool(name="krw", bufs=4) as wpool, \
                 tc.tile_pool(name="krx", bufs=4) as xpool, \
                 tc.tile_pool(name="krev", bufs=10) as evp, \
                 tc.tile_pool(name="kracc", bufs=5, space="PSUM") as accp:
                wts = []
                for m in range(4):
                    wt = wpool.tile([P, HIDC, P], BF16, tag="krw",
                                    name=f"krw{m}")
                    nc.sync.dma_start(out=wt[:], in_=kr_ap[:, :, m * P:(m + 1) * P])
                    wts.append(wt)
                for t in range(ntt if phmax >= 2 else 0):
                    tsl = slice(t * TT, (t + 1) * TT)
                    accs = [accp.tile([P, TT], F32, tag="acc",
                                      name=f"kracc{t}_{m}") for m in range(4)]
                    for k in range(HIDC):
                        xt = xpool.tile([P, TT], BF16, tag="krx",
                                        name=f"krx{t}_{k}")
                        nc.sync.dma_start(out=xt[:], in_=x_ap[:, k, tsl])
                        for mi in range(4):
                            nc.tensor.matmul(
                                accs[mi][:], wts[mi][:, k, :], xt[:],
                                start=(k == 0), stop=(k == HIDC - 1))
                    # chunks [lo0, lo1, hi0, hi1] -> rope
                    for ci in range(2):
                        lo_o, hi_o = rope_evict(accs[ci], accs[2 + ci], tsl,
                                                evp, "ev")
                        rope_store(lo_o, hi_o, ci, krope_d, tsl)

            def load_lat(pool, tag, name, koff, nk, t):
                """Load latent chunks [koff, koff+nk) for token tile t."""
                tiles = []
                src_all = {"q": latq_all, "kv": latkv_all}
                src_nm = {"q": QRC, "kv": KVRC}
                for kk in range(nk):
                    qn = pool.tile([P, TT], BF16, tag=tag,
                                   name=f"{name}{t}_{kk}")
                    for bk in range(4):
                        lo, hi = bk * tta, (bk + 1) * tta
                        if lo >= t * TT and hi <= (t + 1) * TT:
                            nc.sync.dma_start(
                                out=qn[:, lo - t * TT:hi - t * TT],
                                in_=src_all[koff][bk * src_nm[koff] + kk])
                    tiles.append(qn)
                return tiles

            # ------- Phase C: kv_b (k_nope + v) -----------------------------
            # emitted before q_b: it only needs the small kv gather
            with tc.tile_pool(name="kvw", bufs=1) as kvwp, \
                 tc.tile_pool(name="kvn", bufs=2 * KVRC + 1) as kvnp, \
                 tc.tile_pool(name="kvev", bufs=4) as evp, \
                 tc.tile_pool(name="kvps", bufs=4, space="PSUM") as kvps:
                kbw = kvwp.tile([P, KVRC, NH * DN], BF16)
                vbw = kvwp.tile([P, KVRC, NH * DV], BF16)
                nc.sync.dma_start(out=kbw[:], in_=kvbk_ap[:, :, :])
                nc.sync.dma_start(out=vbw[:], in_=kvbv_ap[:, :, :])
                for t in range(ntt if phmax >= 4 else 0):
                    tsl = slice(t * TT, (t + 1) * TT)
                    kvn = load_lat(kvnp, "kvn", "kvn", "kv", KVRC, t)
                    for m in range(NH):
                        ps = kvps.tile([P, TT], F32, tag="kps",
                                       name=f"kb{t}_{m}")
                        for k in range(KVRC):
                            nc.tensor.matmul(ps[:], kbw[:, k, m * P:(m + 1) * P],
                                             kvn[k][:], start=(k == 0),
                                             stop=(k == KVRC - 1))
                        o = evp.tile([P, TT], BF16, tag="ev")
                        nc.vector.tensor_copy(out=o[:], in_=ps[:])
                        nc.sync.dma_start(out=knope_d[:, m, tsl], in_=o[:])
                    for tc8 in range(TT // P):
                        for vc in range(NH * DV // TT):
                            ps = kvps.tile([P, TT], F32, tag="vps",
                                           name=f"v{t}_{tc8}_{vc}")
                            for k in range(KVRC):
                                nc.tensor.matmul(
                                    ps[:],
                                    kvn[k][:, tc8 * P:(tc8 + 1) * P],
                                    vbw[:, k, vc * TT:(vc + 1) * TT],
                                    start=(k == 0), stop=(k == KVRC - 1))
                            o = evp.tile([P, TT], BF16, tag="ev")
                            nc.vector.tensor_copy(out=o[:], in_=ps[:])
                            nc.sync.dma_start(
                                out=v_d[:, t * (TT // P) + tc8,
                                        vc * TT:(vc + 1) * TT],
                                in_=o[:])

            # ------- Phase B: q_b + q rope ----------------------------------
            with tc.tile_pool(name="qbw", bufs=1) as qbwp, \
                 tc.tile_pool(name="qbn", bufs=2 * QRC + 1) as qnp, \
                 tc.tile_pool(name="qbev", bufs=10) as evp, \
                 tc.tile_pool(name="qbps", bufs=8, space="PSUM") as qbps:
                qbw = qbwp.tile([P, QRC, NH * (DN + DR)], BF16)
                nc.sync.dma_start(out=qbw[:], in_=qb_ap[:, :, :])
                for t in range(ntt if phmax >= 3 else 0):
                    tsl = slice(t * TT, (t + 1) * TT)
                    qn = load_lat(qnp, "qn", "qn", "q", QRC, t)
                    rope_ps = {}
                    for m in range(QRC):
                        ps = qbps.tile([P, TT], F32, tag="qbps",
                                       name=f"qb{t}_{m}")
                        for k in range(QRC):
                            nc.tensor.matmul(ps[:], qbw[:, k, m * P:(m + 1) * P],
                                             qn[k][:], start=(k == 0),
                                             stop=(k == QRC - 1))
                        if m < NH:
                            o = evp.tile([P, TT], BF16, tag="ev")
                            nc.vector.tensor_copy(out=o[:], in_=ps[:])
                            nc.sync.dma_start(out=qnope_d[:, m, tsl], in_=o[:])
                        else:
                            rope_ps[m - NH] = ps
                    for ci in range(2):
                        lo_o, hi_o = rope_evict(rope_ps[ci], rope_ps[2 + ci],
                                                tsl, evp, "ev")
                        rope_store(lo_o, hi_o, ci, qrope_d, tsl)

            # ------- Phase D: attention (head pairs, rope row-packed) -------
            nkt = tb // P  # key tiles
            with tc.tile_pool(name="ath", bufs=2) as hp, \
                 tc.tile_pool(name="atex", bufs=6) as exp_p, \
                 tc.tile_pool(name="atev", bufs=4) as evp, \
                 tc.tile_pool(name="atsc", bufs=4, space="PSUM") as scp, \
                 tc.tile_pool(name="atpv", bufs=1, space="PSUM") as pvp, \
                 tc.tile_pool(name="atden", bufs=1, space="PSUM") as denp:
                for j in range(NH // 2 if phmax >= 5 else 0):
                    h0, h1 = 2 * j, 2 * j + 1
                    kn0 = hp.tile([P, tb], BF16, tag="kn0", name=f"kn0_{j}")
                    kn1 = hp.tile([P, tb], BF16, tag="kn1", name=f"kn1_{j}")
                    nc.sync.dma_start(out=kn0[:], in_=knope_d[:, h0, :])
                    nc.sync.dma_start(out=kn1[:], in_=knope_d[:, h1, :])
                    krj = hp.tile([P, tb], BF16, tag="krj", name=f"krj{j}")
                    nc.sync.dma_start(out=krj[:], in_=krope_d[:, j, :])
                    v0 = hp.tile([P, nkt, DV], BF16, tag="v0", name=f"v0_{j}")
                    v1 = hp.tile([P, nkt, DV], BF16, tag="v1", name=f"v1_{j}")
                    nc.sync.dma_start(out=v0[:], in_=v_d[:, :, h0 * DV:(h0 + 1) * DV])
                    nc.sync.dma_start(out=v1[:], in_=v_d[:, :, h1 * DV:(h1 + 1) * DV])
                    qn0 = hp.tile([P, tb], BF16, tag="qn0", name=f"qn0_{j}")
                    qn1 = hp.tile([P, tb], BF16, tag="qn1", name=f"qn1_{j}")
                    nc.sync.dma_start(out=qn0[:], in_=qnope_d[:, h0, :])
                    nc.sync.dma_start(out=qn1[:], in_=qnope_d[:, h1, :])
                    qrj = hp.tile([P, tb], BF16, tag="qrj", name=f"qrj{j}")
                    nc.sync.dma_start(out=qrj[:], in_=qrope_d[:, j, :])
                    for qt in range(ntt):
                        qsl = slice(qt * TT, (qt + 1) * TT)
                        pv0 = pvp.tile([P, TT], F32, tag="pv0",
                                       name=f"pv0_{j}_{qt}")
                        pv1 = pvp.tile([P, TT], F32, tag="pv1",
                                       name=f"pv1_{j}_{qt}")
                        den0 = denp.tile([P, TT], F32, tag="den0",
                                         name=f"den0_{j}_{qt}")
                        den1 = denp.tile([P, TT], F32, tag="den1",
                                         name=f"den1_{j}_{qt}")
                        def pv_den(kt, ex0, ex1):
                            st, sp = kt == 0, kt == nkt - 1
                            nc.tensor.matmul(pv0[:], v0[:, kt, :], ex0[:],
                                             start=st, stop=sp)
                            nc.tensor.matmul(pv1[:], v1[:, kt, :], ex1[:],
                                             start=st, stop=sp)
                            nc.tensor.matmul(den0[:], ones_b[:], ex0[:],
                                             start=st, stop=sp)
                            nc.tensor.matmul(den1[:], ones_b[:], ex1[:],
                                             start=st, stop=sp)

                        # kt loop software-pipelined by one step: PV/den for
                        # kt-1 issue after kt's scores, so the in-order PE
                        # queue never waits on ACT's exp
                        prev = None
                        for kt in range(nkt):
                            ksl = slice(kt * P, (kt + 1) * P)
                            sc0 = scp.tile([P, TT], F32, tag="sc",
                                           name=f"sc0_{j}_{qt}_{kt}")
                            sc1 = scp.tile([P, TT], F32, tag="sc",
                                           name=f"sc1_{j}_{qt}_{kt}")
                            nc.tensor.matmul(sc0[:], kn0[:, ksl], qn0[:, qsl],
                                             start=True, stop=False)
                            nc.tensor.matmul(sc1[:], kn1[:, ksl], qn1[:, qsl],
                                             start=True, stop=False)
                            # K=64 rope matmuls: disjoint row groups run
                            # concurrently in the PE array (tile_position)
                            nc.tensor.matmul(sc0[:], krj[0:64, ksl],
                                             qrj[0:64, qsl],
                                             start=False, stop=True,
                                             tile_position=(0, 0))
                            nc.tensor.matmul(sc1[:], krj[64:128, ksl],
                                             qrj[64:128, qsl],
                                             start=False, stop=True,
                                             tile_position=(64, 0))
                            ex0 = exp_p.tile([P, TT], BF16, tag="ex",
                                             name=f"ex0_{j}_{qt}_{kt}")
                            ex1 = exp_p.tile([P, TT], BF16, tag="ex",
                                             name=f"ex1_{j}_{qt}_{kt}")
                            nc.scalar.activation(ex0[:], sc0[:], EXP_FN,
                                                 scale=SCALE)
                            nc.scalar.activation(ex1[:], sc1[:], EXP_FN,
                                                 scale=SCALE)
                            if prev is not None:
                                pv_den(*prev)
                            prev = (kt, ex0, ex1)
                        pv_den(*prev)
                        for h, pv, den in ((h0, pv0, den0), (h1, pv1, den1)):
                            recip = evp.tile([P, TT], F32, tag="evr",
                                             name="recip")
                            nc.vector.reciprocal(recip[:], den[:])
                            ao = evp.tile([P, TT], BF16, tag="ev", name="ao")
                            nc.vector.tensor_mul(out=ao[:], in0=pv[:],
                                                 in1=recip[:])
                            nc.sync.dma_start(out=attn_d[:, h, qsl], in_=ao[:])

            # ------- Phase E: o_proj (partial) ------------------------------
            with tc.tile_pool(name="oww", bufs=1) as owp, \
                 tc.tile_pool(name="oin", bufs=3) as inp, \
                 tc.tile_pool(name="oev", bufs=3) as evp, \
                 tc.tile_pool(name="ops", bufs=8, space="PSUM") as ops:
                oww = owp.tile([P, NH * DV // P, HID], BF16)
                nc.sync.dma_start(out=oww[:], in_=ow_ap[:, :, :])
                for t8 in range(ntc if phmax >= 6 else 0):
                    at = inp.tile([P, NH, P], BF16, tag="at", name=f"at{t8}")
                    nc.sync.dma_start(out=at[:],
                                      in_=attn_d[:, :, t8 * P:(t8 + 1) * P])
                    for n in range(HID // TT):
                        ps = ops.tile([P, TT], F32, tag="ops", name=f"o{t8}_{n}")
                        for k in range(NH * DV // P):
                            nc.tensor.matmul(ps[:], at[:, k, :],
                                             oww[:, k, n * TT:(n + 1) * TT],
                                             start=(k == 0),
                                             stop=(k == NH * DV // P - 1))
                        o = evp.tile([P, TT], F32, tag="ev")
                        nc.vector.tensor_copy(out=o[:], in_=ps[:])
                        nc.sync.dma_start(
                            out=out_part[t8 * P:(t8 + 1) * P,
                                         n * TT:(n + 1) * TT],
                            in_=o[:])

    nc.compile()
    return nc


# ---------------------------------------------------------------------------
# host-side packing
# ---------------------------------------------------------------------------

def _rope_tables():
    inv_freq = 1.0 / (10000.0 ** (np.arange(0, DR, 2, dtype=np.float32) / DR))
    t = np.arange(MAXP, dtype=np.float32)
    freqs = np.outer(t, inv_freq)
    emb = np.concatenate([freqs, freqs], axis=-1)
    return np.cos(emb).astype(np.float32), np.sin(emb).astype(np.float32)


def core_weights(g, q_a_w, q_a_ln_w, q_b_w, kv_a_w, kv_a_ln_w, kv_b_w,
                 k_rope_w, o_w):
    """Pack the weight set for head-group g (heads g*8 .. g*8+8)."""
    heads = range(g * NH, (g + 1) * NH)
    qb_eff = (q_b_w * q_a_ln_w[None, :]).astype(np.float32)
    kvb_eff = (kv_b_w * kv_a_ln_w[None, :]).astype(np.float32)

    nope_rows = np.concatenate(
        [np.arange(h * (DN + DR), h * (DN + DR) + DN) for h in heads])
    lo_rows = np.concatenate(
        [np.arange(h * (DN + DR) + DN, h * (DN + DR) + DN + 32) for h in heads])
    hi_rows = np.concatenate(
        [np.arange(h * (DN + DR) + DN + 32, h * (DN + DR) + DN + 64)
         for h in heads])
    qb_rows = np.concatenate([nope_rows, lo_rows, hi_rows])

    k_rows = np.concatenate(
        [np.arange(h * (DN + DV), h * (DN + DV) + DN) for h in heads])
    v_rows = np.concatenate(
        [np.arange(h * (DN + DV) + DN, (h + 1) * (DN + DV)) for h in heads])

    kr_lo = np.concatenate([np.arange(h * DR, h * DR + 32) for h in heads])
    kr_hi = np.concatenate([np.arange(h * DR + 32, (h + 1) * DR) for h in heads])
    kr_rows = np.concatenate([kr_lo, kr_hi])

    o_cols = np.concatenate([np.arange(h * DV, (h + 1) * DV) for h in heads])

    def c(a):
        return np.ascontiguousarray(a).astype(NP_BF16)

    return {
        "qa_wT": c(q_a_w.T),
        "kva_wT": c(kv_a_w.T),
        "kr_wT": c(k_rope_w[kr_rows].T),
        "qb_wT": c(qb_eff[qb_rows].T),
        "kvbk_wT": c(kvb_eff[k_rows].T),
        "kvbv_wT": c(kvb_eff[v_rows].T),
        "o_wT": c(o_w[:, o_cols].T),
    }


def core_inputs(c, hidden_states, position_ids, weight_sets, tb=S,
                seqpar=SEQPAR):
    b, g = c // 4, c % 4
    tta = tb // 4
    cos_tab, sin_tab = _rope_tables()
    pos = np.asarray(position_ids[b][:tb])
    cos_b = cos_tab[pos]  # [tb, DR]
    sin_b = sin_tab[pos]
    cos_rep = np.ascontiguousarray(np.tile(cos_b[:, :32].T, (4, 1)))
    sin_rep = np.ascontiguousarray(np.tile(sin_b[:, :32].T, (4, 1)))
    x = np.asarray(hidden_states[b][:tb], dtype=np.float32)
    xTb = np.ascontiguousarray(x.T).astype(NP_BF16)
    im = {"xT": xTb, "cos_rep": cos_rep, "sin_rep": sin_rep}
    if seqpar:
        im["xA"] = np.ascontiguousarray(xTb[:, g * tta:(g + 1) * tta])
    im.update(weight_sets[g])
    return im


_CACHE = {}


def _get_nc(tb=S):
    if tb not in _CACHE:
        _CACHE[tb] = build_nc(tb)
    return _CACHE[tb]


def kernel(hidden_states, position_ids, q_a_w, q_a_ln_w, q_b_w,
           kv_a_w, kv_a_ln_w, kv_b_w, k_rope_w, o_w):
    hidden_states = np.asarray(hidden_states, dtype=np.float32)
    weight_sets = [
        core_weights(g, np.asarray(q_a_w, np.float32),
                     np.asarray(q_a_ln_w, np.float32),
                     np.asarray(q_b_w, np.float32),
                     np.asarray(kv_a_w, np.float32),
                     np.asarray(kv_a_ln_w, np.float32),
                     np.asarray(kv_b_w, np.float32),
                     np.asarray(k_rope_w, np.float32),
                     np.asarray(o_w, np.float32))
        for g in range(4)
    ]
    in_maps = [core_inputs(c, hidden_states, position_ids, weight_sets)
               for c in range(NCORES)]
    nc = _get_nc()
    res = run_bass_kernel_spmd(nc, in_maps, core_ids=list(range(NCORES)))
    out = np.zeros((B, S, HID), dtype=np.float64)
    for c in range(NCORES):
        out[c // 4] += res.results[c]["out_part"]
    return out.astype(np.float32)
